# revision 21
# baseline (speedup 1.0000x reference)
"""Causal self-attention Trainium2 kernel (B=8, S=1024, C=768, H=12).

Sharding: pure data-parallel over batch - core i computes batch i end-to-end.
No collectives. Weights are replicated to all 8 cores.

Design notes (vs the original baseline):
  - PE p-state: TRN2 PE runs at 1.2 GHz until it has been continuously busy
    ~3us, then 2.4 GHz. The schedule keeps the PE stream dense: logits run
    one head ahead of AV, and V/QK projection chunks are spliced into the
    scalar-bound early attention phases as filler work.
  - ScalarE does exp ONLY, batched into [128,1024] activations (72 total).
  - GpSimd: diagonal tri-masking via affine_select (SBUF-only engine).
  - DVE: projection bias evacuations, AV evacuation, reciprocal, y
    normalize. All engine ops keep input/output on the same partitions;
    V is stored padded to 128 stationary columns with the ones column at
    position 64 (even heads) / 63 (odd heads) so odd heads' AV lands on
    partitions 64:128 directly.
  - Softmax denominator via the fused ones-column (extra AV psum row).
  - All matmuls fp16 inputs (fp8 was tested and fails the accuracy budget:
    attention output is a random-sign average, elementwise noise does not
    average out).
"""

import sys
import types

import numpy as np

import concourse.bass as bass
import concourse.mybir as mybir
import concourse.tile as tile
from concourse import bacc


def _ensure_axon_hooks():
    """The container's `antenv` stub lacks `axon_hooks`, which
    run_bass_kernel_spmd imports when trace=True under axon. Provide it and
    register the NTFF profile hook so tracing works."""
    try:
        import antenv.axon_hooks  # noqa: F401

        return
    except ImportError:
        pass
    try:
        import antenv
    except ImportError:
        return
    mod = types.ModuleType("antenv.axon_hooks")
    _store = [None]
    mod.set_axon_ntff_profile_hook = lambda h: _store.__setitem__(0, h)
    mod.get_axon_ntff_profile_hook = lambda: _store[0]
    sys.modules["antenv.axon_hooks"] = mod
    antenv.axon_hooks = mod
    try:
        from trn_agent_boot.trn_boot import _ntff_profile_via_ctypes

        hook = _ntff_profile_via_ctypes("/opt/axon/libaxon_pjrt.so")
        mod.set_axon_ntff_profile_hook(hook)
    except Exception:
        pass


_ensure_axon_hooks()

P = 128
C = 768
H = 12
D = 64
NT_C = C // P          # 6 c-tiles
QC = 256               # attention q-chunk
F32 = mybir.dt.float32
F16 = mybir.dt.float16


def build_nc(S=1024):
    NQC = S // QC          # 4 q-chunks
    NT_S = S // P          # 8 s-tiles

    nc = bacc.Bacc("TRN2", target_bir_lowering=False, debug=False)

    xt_d = nc.dram_tensor("xt", [C, S], F16, kind="ExternalInput")
    wqk_d = nc.dram_tensor("wqkT", [C, 2 * C], F16, kind="ExternalInput")
    wv_d = nc.dram_tensor("wvT", [C, C], F16, kind="ExternalInput")
    wo_d = nc.dram_tensor("woutT", [C, C], F16, kind="ExternalInput")
    bqk_d = nc.dram_tensor("bqk", [2 * C], F32, kind="ExternalInput")
    bv_d = nc.dram_tensor("bv", [C], F32, kind="ExternalInput")
    bo_d = nc.dram_tensor("bout", [C], F32, kind="ExternalInput")
    out_d = nc.dram_tensor("out", [S, C], F32, kind="ExternalOutput")

    with tile.TileContext(nc) as tc:
        with (
            tc.tile_pool(name="lg_ps", bufs=3, space="PSUM") as lg_ps,
            tc.tile_pool(name="av_ps", bufs=2, space="PSUM") as av_ps,
            tc.tile_pool(name="const", bufs=1) as cpool,
            tc.tile_pool(name="big", bufs=1) as gpool,
            tc.tile_pool(name="pt", bufs=4) as ptpool,
            tc.tile_pool(name="yu", bufs=16) as yupool,
            tc.tile_pool(name="dn", bufs=2) as dnpool,
            tc.tile_pool(name="bc", bufs=4) as bcpool,
            tc.tile_pool(name="rc", bufs=4) as rcpool,
            tc.tile_pool(name="evac", bufs=3) as epool,
        ):
            # ---------------- constants / inputs ----------------
            bqk_sb = cpool.tile([P, 2 * NT_C], F32)
            nc.scalar.dma_start(bqk_sb[:], bqk_d[:].rearrange("(t p) -> p t", p=P))
            bv_bc = cpool.tile([P, C], F32)
            nc.scalar.dma_start(bv_bc[:], bv_d[:][None, :].to_broadcast((P, C)))
            bo_bc = cpool.tile([P, C], F32)
            nc.scalar.dma_start(bo_bc[:], bo_d[:][None, :].to_broadcast((P, C)))

            xt_sb = gpool.tile([P, NT_C, S], F16)
            xt_r = xt_d[:, :].rearrange("(ct p) s -> p ct s", p=P)
            for ct in range(NT_C):
                nc.sync.dma_start(xt_sb[:, ct, :], xt_r[:, ct, :])

            wqk_sb = gpool.tile([P, NT_C, 2 * C], F16)
            wqk_r = wqk_d[:, :].rearrange("(ct p) n -> p ct n", p=P)
            for ct in range(NT_C):
                nc.scalar.dma_start(wqk_sb[:, ct, :], wqk_r[:, ct, :])

            wv_sb = gpool.tile([P, NT_C, C], F16)
            wv_r = wv_d[:, :].rearrange("(ct p) n -> p ct n", p=P)
            for ct in range(NT_C):
                nc.scalar.dma_start(wv_sb[:, ct, :], wv_r[:, ct, :])

            wo_sb = gpool.tile([P, NT_C, C], F16)
            wo_r = wo_d[:, :].rearrange("(ct p) n -> p ct n", p=P)
            for ct in range(NT_C):
                nc.scalar.dma_start(wo_sb[:, ct, :], wo_r[:, ct, :])

            # persistent activations
            qk_sb = gpool.tile([P, 2 * NT_C, S], F16)   # Q tiles 0..5, K 6..11
            # V with fused ones column: av rows 0:64 = y, row 64 = denominator.
            # Odd heads' y is moved to partitions 64:128 later by DMA (engine
            # ops cannot cross partitions).
            vp_sb = gpool.tile([P, NT_S, H, D + 1], F16)  # [s, st, h, d|1]
            nc.gpsimd.memset(vp_sb[:, :, :, D : D + 1], 1.0)
            y_sb = gpool.tile([P, NT_C, S], F16)

            # ---------------- projection emitters ----------------
            def emit_qk_tile(t, sb):
                """qk_sb[:, t, sb*512:(sb+1)*512]"""
                ps = lg_ps.tile([P, 1024], F32, tag="lg")
                for ct in range(NT_C):
                    nc.tensor.matmul(
                        ps[:, :512],
                        wqk_sb[:, ct, t * P : (t + 1) * P],
                        xt_sb[:, ct, sb * 512 : (sb + 1) * 512],
                        start=(ct == 0),
                        stop=(ct == NT_C - 1),
                    )
                nc.vector.tensor_scalar_add(
                    qk_sb[:, t, sb * 512 : (sb + 1) * 512],
                    ps[:, :512],
                    bqk_sb[:, t : t + 1],
                )

            def emit_v_tile(st):
                """vp_sb[:, st, :, :] (+bias) with head-parity column layout."""
                for cs, cw in ((0, 512), (512, 256)):
                    ps = lg_ps.tile([P, 1024], F32, tag="lg")
                    for ct in range(NT_C):
                        nc.tensor.matmul(
                            ps[:, :cw],
                            xt_sb[:, ct, st * P : (st + 1) * P],
                            wv_sb[:, ct, cs : cs + cw],
                            start=(ct == 0),
                            stop=(ct == NT_C - 1),
                        )
                    nh = cw // D
                    h0 = cs // D
                    nc.vector.tensor_add(
                        vp_sb[:, st, h0 : h0 + nh, 0:D],
                        ps[:, :cw].rearrange("p (h d) -> p h d", d=D),
                        bv_bc[:, cs : cs + cw].rearrange("p (h d) -> p h d", d=D),
                    )

            def emit_outproj(qc):
                """out rows for s-tiles 2qc, 2qc+1 + bias + DMA."""
                for st in (2 * qc, 2 * qc + 1):
                    ot = epool.tile([P, C], F32, tag="ot")
                    for cs, cw in ((0, 512), (512, 256)):
                        ps = lg_ps.tile([P, 1024], F32, tag="lg")
                        for ct in range(NT_C):
                            nc.tensor.matmul(
                                ps[:, :cw],
                                y_sb[:, ct, st * P : (st + 1) * P],
                                wo_sb[:, ct, cs : cs + cw],
                                start=(ct == 0),
                                stop=(ct == NT_C - 1),
                            )
                        nc.vector.tensor_add(
                            ot[:, cs : cs + cw], ps[:, :cw], bo_bc[:, cs : cs + cw]
                        )
                    nc.sync.dma_start(out_d[st * P : (st + 1) * P, :], ot[:])

            # ---------------- attention emitters ----------------
            def emit_logits(h, qc):
                """lg psum tiles for head h, q-chunk qc. Tile k holds jp pairs
                (2k, 2k+1), 512 cols each; the diagonal pair jp==qc is last.
                The all-masked lower half of the odd diagonal key tile is
                computed anyway (cheaper than splitting the exp) and zeroed
                by the mask; AV never reads it."""
                pair = h // 2
                lo = (h % 2) * D
                hi = lo + D
                q0 = qc * QC
                tiles = []
                for jp in range(qc + 1):
                    slot = jp % 2
                    if slot == 0:
                        lgT = lg_ps.tile([P, 1024], F32, tag="lg",
                                         name=f"lg_{qc}_{h}_{jp}")
                        tiles.append(lgT)
                    for dj in range(2):
                        j = 2 * jp + dj
                        nc.tensor.matmul(
                            lgT[:, slot * 512 + dj * QC : slot * 512 + (dj + 1) * QC],
                            qk_sb[lo:hi, 6 + pair, j * P : (j + 1) * P],
                            qk_sb[lo:hi, pair, q0 : q0 + QC],
                            start=True,
                            stop=True,
                            skip_group_check=True,
                        )
                return tiles

            def emit_exp(h, qc, tiles):
                """exp of all logits into one SBUF pt tile [P,(qc+1)*512]."""
                pt = ptpool.tile([P, 2048], F16, tag="pt", name=f"pt_{qc}_{h}")
                n = qc + 1  # jp pairs
                done = 0
                for lgT in tiles:
                    w = min(2, n - done) * 512
                    nc.scalar.activation(
                        pt[:, done * 512 : done * 512 + w],
                        lgT[:, :w],
                        mybir.ActivationFunctionType.Exp,
                        scale=0.125,
                    )
                    done += w // 512
                return pt

            def emit_mask(pt, qc):
                """Tri-mask the diagonal pair (cols qc*512 + [0:128 | 384:512]):
                keep q_local >= k_local (partition), else 0."""
                base = qc * 512
                view = pt[:, base : base + 512].rearrange(
                    "p (a q) -> p a q", a=4
                )[:, ::3, :]
                nc.gpsimd.affine_select(
                    out=view,
                    in_=view,
                    compare_op=mybir.AluOpType.is_ge,
                    fill=0.0,
                    base=0,
                    pattern=[[0, 2], [1, P]],
                    channel_multiplier=-1,
                )

            def emit_av(h, qc, pt):
                ncol = D + 1
                avp = av_ps.tile([P, QC], F32, tag="av", name=f"av_{qc}_{h}")
                out = avp[0:ncol, :]
                for jp in range(qc):
                    for dj in range(2):
                        nc.tensor.matmul(
                            out,
                            vp_sb[:, 2 * jp + dj, h, 0:ncol],
                            pt[:, (2 * jp + dj) * QC : (2 * jp + dj + 1) * QC],
                            start=(jp == 0 and dj == 0),
                            stop=False,
                            skip_group_check=True,
                        )
                # diagonal pair: cols 0:128 <- j0 only; cols 128:256 <- j0+j1
                j0, j1 = 2 * qc, 2 * qc + 1
                db = 2 * qc * QC
                nc.tensor.matmul(
                    out[:, 0:P],
                    vp_sb[:, j0, h, 0:ncol],
                    pt[:, db : db + P],
                    start=(qc == 0),
                    stop=True,
                    skip_group_check=True,
                )
                nc.tensor.matmul(
                    out[:, P:QC],
                    vp_sb[:, j0, h, 0:ncol],
                    pt[:, db + P : db + QC],
                    start=(qc == 0),
                    stop=False,
                    skip_group_check=True,
                )
                nc.tensor.matmul(
                    out[:, P:QC],
                    vp_sb[:, j1, h, 0:ncol],
                    pt[:, db + 3 * P : db + 4 * P],
                    start=False,
                    stop=True,
                    skip_group_check=True,
                )
                return avp

            def emit_evac(h, qc, avp, dnq):
                yu = yupool.tile([D + 1, QC], F16, tag="yu", name=f"yu_{qc}_{h}")
                nc.vector.tensor_copy(yu[0 : D + 1, :], avp[0 : D + 1, :])
                nc.sync.dma_start(dnq[h : h + 1, :], yu[D : D + 1, :])
                return yu

            def emit_norm(h, qc, yu, dnq):
                pair = h // 2
                rc = rcpool.tile([1, QC], F16, tag="rc", name=f"rc_{qc}_{h}")
                nc.sync.dma_start(rc[:], dnq[h : h + 1, :])
                bc = bcpool.tile([D, QC], F16, tag="bc")
                nc.gpsimd.partition_broadcast(bc[:], rc[:])
                if h % 2 == 0:
                    nc.vector.tensor_mul(
                        y_sb[0:D, pair, qc * QC : (qc + 1) * QC],
                        yu[0:D, :],
                        bc[:],
                    )
                else:
                    # normalize in place, then DMA across partitions
                    nc.vector.tensor_mul(yu[0:D, :], yu[0:D, :], bc[:])
                    nc.sync.dma_start(
                        y_sb[D:P, pair, qc * QC : (qc + 1) * QC], yu[0:D, :]
                    )

            # ---------------- schedule ----------------
            # Lead-in: K tiles then Q tiles for cols 0:512, V s-tiles 0,1.
            for t in (6, 7, 8, 9, 10, 11, 0, 1, 2, 3, 4, 5):
                emit_qk_tile(t, 0)
            emit_v_tile(0)
            emit_v_tile(1)

            # filler emitted between attention heads of qc 0/1 to keep the
            # PE dense while ScalarE works through the exps
            filler = [("v", 2), ("v", 3), ("v", 4), ("v", 5), ("v", 6), ("v", 7)]
            filler += [("qk", t) for t in (6, 7, 8, 9, 10, 11, 0, 1, 2, 3, 4, 5)]
            filler.reverse()  # pop() from the front order

            def emit_filler(n):
                for _ in range(n):
                    if not filler:
                        return
                    kind, arg = filler.pop()
                    if kind == "v":
                        emit_v_tile(arg)
                    else:
                        emit_qk_tile(arg, 1)

            with nc.allow_low_precision(reason="fp16 softmax denominators"):
                prev = None       # (h, qc, pt) logits+exp+mask done, AV pending
                prev_qc_state = None  # (qc, yus, dnq) awaiting normalize
                prev_dn = None
                for qc in range(NQC):
                    dnq = dnpool.tile([H, QC], F16, tag="dn", name=f"dn_{qc}")
                    yus = []
                    for h in range(H):
                        tiles = emit_logits(h, qc)
                        pt = emit_exp(h, qc, tiles)
                        emit_mask(pt, qc)
                        if prev is not None:
                            ph, pqc, ppt = prev
                            avp = emit_av(ph, pqc, ppt)
                            pdnq, pyus = (dnq, yus) if pqc == qc else prev_dn
                            pyus.append(emit_evac(ph, pqc, avp, pdnq))
                        prev = (h, qc, pt)
                        if qc <= 2:
                            emit_filler(1)
                        # one head into qc, the previous qc's AVs are done:
                        # normalize + out-project the previous q-chunk
                        if h == 1 and prev_qc_state is not None:
                            nqc, nyus, ndnq = prev_qc_state
                            nc.vector.reciprocal(ndnq[:], ndnq[:])
                            for nh in range(H):
                                emit_norm(nh, nqc, nyus[nh], ndnq)
                            prev_qc_state = None
                        if h == 3 and qc > 0:
                            emit_outproj(qc - 1)
                    prev_dn = (dnq, yus)
                    prev_qc_state = (qc, yus, dnq)

                # drain: last head's AV, last q-chunk normalize + outproj
                ph, pqc, ppt = prev
                avp = emit_av(ph, pqc, ppt)
                yus.append(emit_evac(ph, pqc, avp, dnq))
                nqc, nyus, ndnq = prev_qc_state
                nc.vector.reciprocal(ndnq[:], ndnq[:])
                for nh in range(H):
                    emit_norm(nh, nqc, nyus[nh], ndnq)
                emit_outproj(NQC - 1)

    nc.compile()
    return nc


_NC_CACHE = {}


def _get_nc(S):
    if S not in _NC_CACHE:
        _NC_CACHE[S] = build_nc(S)
    return _NC_CACHE[S]


def make_in_maps(x, w_qkv, b_qkv, w_out, b_out):
    x = np.asarray(x, np.float32)
    w_qkv = np.asarray(w_qkv, np.float32)
    b_qkv = np.asarray(b_qkv, np.float32)
    w_out = np.asarray(w_out, np.float32)
    b_out = np.asarray(b_out, np.float32)
    B = x.shape[0]
    xt = np.ascontiguousarray(x.transpose(0, 2, 1)).astype(np.float16)
    wqkT = np.ascontiguousarray(w_qkv[: 2 * C].T).astype(np.float16)
    wvT = np.ascontiguousarray(w_qkv[2 * C :].T).astype(np.float16)
    woT = np.ascontiguousarray(w_out.T).astype(np.float16)
    bqk = np.ascontiguousarray(b_qkv[: 2 * C])
    bv = np.ascontiguousarray(b_qkv[2 * C :])
    bo = np.ascontiguousarray(b_out)
    return [
        {
            "xt": xt[i],
            "wqkT": wqkT,
            "wvT": wvT,
            "woutT": woT,
            "bqk": bqk,
            "bv": bv,
            "bout": bo,
        }
        for i in range(B)
    ]


def kernel_with_results(x, w_qkv, b_qkv, w_out, b_out, attention_mask=None, **run_kw):
    from concourse.bass_utils import run_bass_kernel_spmd

    B, S, C_ = x.shape
    assert C_ == C
    nc = _get_nc(S)
    in_maps = make_in_maps(x, w_qkv, b_qkv, w_out, b_out)
    res = run_bass_kernel_spmd(nc, in_maps, core_ids=list(range(B)), **run_kw)
    out = np.stack([m["out"] for m in res.results], axis=0).astype(np.float32)
    return out, res


def kernel(x, w_qkv, b_qkv, w_out, b_out, attention_mask=None):
    out, _ = kernel_with_results(x, w_qkv, b_qkv, w_out, b_out, attention_mask)
    return out
